# revision 10
# baseline (speedup 1.0000x reference)
"""CenterLoss kernel for 8 Trainium2 NeuronCores.

loss = mean(distmat * onehot(labels)) over a (B, C) distmat where
distmat[i, j] = ||x_i - c_j||^2.  The mask selects exactly one element
per row, so  loss = (1/(B*C)) * sum_i ||x_i - c_{labels[i]}||^2.

Strategy: data-parallel over batch.  Each of the 8 cores takes 512 rows
of x, gathers its 512 center rows from the (replicated) centers table
with an indirect DMA, computes sum((x-g)^2) on the vector engine, and
writes a [128,1] per-partition partial sum.  The host sums the 8x128
partials in float64 and divides by B*C.

Raw Bass (no Tile): the toolchain allows at most one semaphore wait
per compute instruction, so cross-engine deps are taken with
standalone wait_ge instructions instead of instruction-attached waits.
"""

import sys

if "/opt/trn_rl_repo" not in sys.path:
    sys.path.insert(0, "/opt/trn_rl_repo")

import numpy as np

import concourse.bass as bass
from concourse import mybir

NCORES = 8
B = 4096
D = 128
C = 20000
P = 128
BS = B // NCORES          # 512 rows per core
N = BS // P               # 4 rows per partition


def build_bass() -> bass.Bass:
    nc = bass.Bass()
    x = nc.declare_dram_parameter("x", [BS, D], mybir.dt.float32, isOutput=False)
    idx = nc.declare_dram_parameter("idx", [BS], mybir.dt.int32, isOutput=False)
    centers = nc.declare_dram_parameter(
        "centers", [C, D], mybir.dt.float32, isOutput=False
    )
    out = nc.declare_dram_parameter("out", [P, 1], mybir.dt.float32, isOutput=True)

    with (
        nc.sbuf_tensor([P, N], mybir.dt.int32) as idx_t,
        nc.sbuf_tensor([P, N * D], mybir.dt.float32) as x_t,
        nc.sbuf_tensor([P, N * D], mybir.dt.float32) as g_t,
        nc.sbuf_tensor([P, N * D], mybir.dt.float32) as d_t,
        nc.sbuf_tensor([P, N * D], mybir.dt.float32) as sq_t,
        nc.sbuf_tensor([P, 1], mybir.dt.float32) as red_t,
        nc.semaphore("idx_sem") as idx_sem,
        nc.semaphore("x_sem") as x_sem,
        nc.semaphore("g_sem") as g_sem,
        nc.semaphore("v_sem") as v_sem,
        nc.semaphore("done_sem") as done_sem,
        nc.Block() as block,
    ):

        @block.sync
        def _(sync):
            sync.dma_start(
                out=idx_t[:], in_=idx[:].rearrange("(p n) -> p n", p=P)
            ).then_inc(idx_sem, 16)
            sync.dma_start(
                out=x_t[:], in_=x[:].rearrange("(p n) d -> p (n d)", p=P)
            ).then_inc(x_sem, 16)
            sync.wait_ge(v_sem, 3)
            sync.dma_start(out=out[:], in_=red_t[:]).then_inc(done_sem, 16)
            sync.wait_ge(done_sem, 16)

        @block.gpsimd
        def _(gpsimd):
            gpsimd.wait_ge(idx_sem, 16)
            # HW honors only one offset per partition per indirect DMA, so
            # issue N gathers with [P, 1] offset tiles.
            for n in range(N):
                gpsimd.indirect_dma_start(
                    out=g_t[:, n * D : (n + 1) * D],
                    out_offset=None,
                    in_=centers[:],
                    in_offset=bass.IndirectOffsetOnAxis(
                        ap=idx_t[:, n : n + 1], axis=0
                    ),
                ).then_inc(g_sem, 16)

        @block.vector
        def _(vector):
            vector.wait_ge(x_sem, 16)
            vector.wait_ge(g_sem, 16 * N)
            # The chain sems between DVE ops are free on HW (they overlap
            # the per-op pipeline DRAIN) and keep the race detector happy.
            vector.tensor_tensor(
                out=d_t[:], in0=x_t[:], in1=g_t[:], op=mybir.AluOpType.subtract
            ).then_inc(v_sem, 1)
            vector.wait_ge(v_sem, 1)
            vector.tensor_tensor(
                out=sq_t[:], in0=d_t[:], in1=d_t[:], op=mybir.AluOpType.mult
            ).then_inc(v_sem, 1)
            vector.wait_ge(v_sem, 2)
            vector.tensor_reduce(
                out=red_t[:],
                in_=sq_t[:],
                axis=mybir.AxisListType.X,
                op=mybir.AluOpType.add,
            ).then_inc(v_sem, 1)

    return nc


_NC = None


def _get_nc() -> bass.Bass:
    global _NC
    if _NC is None:
        _NC = build_bass()
    return _NC


def make_in_maps(x, labels, centers):
    x = np.ascontiguousarray(np.asarray(x, dtype=np.float32))
    labels = np.asarray(labels).astype(np.int32)
    centers = np.ascontiguousarray(np.asarray(centers, dtype=np.float32))
    in_maps = []
    for c in range(NCORES):
        sl = slice(c * BS, (c + 1) * BS)
        in_maps.append(
            {
                "x": np.ascontiguousarray(x[sl]),
                "idx": np.ascontiguousarray(labels[sl]),
                "centers": centers,
            }
        )
    return in_maps


def reduce_outputs(results) -> np.ndarray:
    total = 0.0
    for r in results:
        total += float(np.sum(r["out"].astype(np.float64)))
    return np.array(np.float32(total / (B * C)))


def kernel(x, labels, centers) -> np.ndarray:
    from concourse.bass_utils import run_bass_kernel_spmd

    nc = _get_nc()
    in_maps = make_in_maps(x, labels, centers)
    res = run_bass_kernel_spmd(nc, in_maps, list(range(NCORES)))
    return reduce_outputs(res.results)
